# revision 4
# baseline (speedup 1.0000x reference)
"""2-layer GAT (GATNet) forward on 8 Trainium2 NeuronCores via Bass/Tile.

v2: minimizes host->device bytes (the axon tunnel dominates wall time) and
drops both per-edge a_dst gathers.

Sharding: 128 graphs -> 16 per core, slot layout: graph g gets L padded
slots; core c owns global slot rows [c*SL, (c+1)*SL).  ALL tables are in
slot order, so ONE edge-src index table serves both layers.

Host ships per core only: x stripe [F, SL] (bf16), edge src slot ids
(compact 16-partition wrap), per-block dst-local columns (bf16), phantom
mask, and the raw weights (bf16 W1/W2 + f32 att/b/fc).  Everything else
(iota, identities, transposed weights, index replication) is built on
device.  x stripes are AllGathered on device; phase B (h_ext for all
slots) is replicated.

Layer 1: h_ext = x @ [W1 | W1@att_src1] for ALL slots -> DRAM table
hx ([h bf16 | a_src f32]).  a_dst is NOT per-edge-gathered: it only
depends on the dst slot, so each core computes adw[q, w] = x_my @
(W1@att_dst1) per window (tiny matmuls) and each edge block derives its
per-edge a_dst with S2 = PE-transpose(S) and one [128x128]x[128xH]
matmul.  ex = exp(leakyrelu(a_src + a_dst)); messages scaled in place;
0/1 selection matrix S[e, dst_local] turns the per-128-dst-window
segmented softmax sums into PE matmuls accumulated in PSUM.

Layer 2: h2_ext = elu1 @ [W2 | W2@att_src2 | W2@att_dst2] on local slots,
AllGather, same edge pass with a single head (a_dst2 kept in SBUF).

Pooling: phantom slots masked to -1e30, tensor_reduce(max) over
[128, GPC, L], FC + ReLU; each core outputs its 16 graphs [16, 128].
"""
import sys
import numpy as np

for _p in ("/opt/trn_rl_repo", "/root/.axon_site/_ro/trn_rl_repo"):
    if _p not in sys.path:
        sys.path.append(_p)

import json as _json
from contextlib import ExitStack

import concourse.bass as bass
import concourse.mybir as mybir
import concourse.tile as tile
import bass_rust as _bass_rust
import concourse.bass_utils as _bass_utils
import concourse.bass2jax as _bass2jax
from concourse.library_config import all_libraries as _all_libs, standard as _std_lib

F32 = mybir.dt.float32
BF16 = mybir.dt.bfloat16
I16 = mybir.dt.int16
AF = mybir.ActivationFunctionType
OP = mybir.AluOpType

NC = 8
NEG_SLOPE = 0.2
EPS = 1e-6
NEG_BIG = -1.0e30
CH = 8           # gather chunk size in 128-edge blocks
DMA_SCRATCH = 16384   # SWDGE descriptor carveout: //16 = 1024 descriptors

# ------------------------------------------------------------- walrus fixups

_orig_compile_bir_kernel = _bass_utils.compile_bir_kernel


def _split_multiwaits(j):
    """This walrus build encodes at most ONE sync-wait per instruction;
    move extra waits onto NoOp carriers."""
    n = 0
    for f in j.get("functions", []):
        for bb in f.get("blocks", []):
            insts = bb.get("instructions", [])
            if not any(
                len(((i.get("sync_info") or {}).get("on_wait") or [])) > 1
                for i in insts
            ):
                continue
            new = []
            for i in insts:
                si = i.get("sync_info")
                w = (si or {}).get("on_wait") or []
                if len(w) > 1:
                    for extra in w[:-1]:
                        n += 1
                        new.append({
                            "debug": i.get("debug", 0),
                            "engine": i["engine"],
                            "ins": [], "outs": [],
                            "name": f"I-mws-{n}",
                            "opcode": "NoOp",
                            "sync_info": {"on_update": [], "on_wait": [extra]},
                        })
                    si["on_wait"] = [w[-1]]
                new.append(i)
            bb["instructions"] = new
    return j


def _patched_compile_bir_kernel(bir_json, tmpdir, neff_name="file.neff"):
    j = _json.loads(bir_json)
    j = _split_multiwaits(j)
    return _orig_compile_bir_kernel(
        _json.dumps(j).encode(), tmpdir, neff_name=neff_name)


def apply_patches():
    _bass_utils.compile_bir_kernel = _patched_compile_bir_kernel
    _bass2jax.compile_bir_kernel = _patched_compile_bir_kernel


def finalize_program(nc):
    """Bacc-style post passes that raw Bass/Tile skips: insert gpsimd
    library loads and encode extended-ISA instruction words."""
    mask = {}
    for lib in _all_libs:
        for it in lib.instructions:
            mask[it] = mask.get(it, 0) | (1 << lib.index)
    _bass_rust.insert_library_loads(nc, mask, len(_all_libs), _std_lib.index)
    mybir.codegen_inst_isa_subclasses(nc)


# ------------------------------------------------------------- host prep

def _wrap16(idx):
    """dma_gather idx layout, compact: idx i -> partition i%16, slot i//16.
    [n] -> [16, n//16].  Replicated to 128 partitions on device."""
    n = len(idx)
    assert n % 16 == 0
    return idx.reshape(n // 16, 16).T.astype(np.int16).copy()


def host_prep(x, edge_index, batch):
    import ml_dtypes
    N, F = x.shape
    G = int(np.asarray(batch).max()) + 1
    assert G % NC == 0, f"graphs {G} not divisible by {NC}"
    GPC = G // NC

    src = np.concatenate([np.asarray(edge_index[0], np.int64),
                          np.arange(N, dtype=np.int64)])
    dst = np.concatenate([np.asarray(edge_index[1], np.int64),
                          np.arange(N, dtype=np.int64)])

    bat = np.asarray(batch, dtype=np.int64)
    counts = np.bincount(bat, minlength=G)
    start = np.zeros(G + 1, dtype=np.int64)
    np.cumsum(counts, out=start[1:])

    stepmod = 128 // int(np.gcd(GPC, 128))
    L = int(np.ceil(max(1, counts.max()) / stepmod) * stepmod)
    SL = GPC * L
    W = SL // 128
    SLP = SL + 16          # per-core row stride in hx/h2x (16 pad rows)
    NROWX = NC * SLP
    assert SL % 128 == 0
    assert NROWX <= 32766, f"slot rows {NROWX} overflow int16"

    # permute graphs: serpentine-deal by edge count so the k-th graph of
    # every core has a similar profile -> less per-window max padding
    ecnt = np.bincount(bat[dst], minlength=G)
    order = np.argsort(-ecnt, kind="stable")
    perm = np.zeros(G, dtype=np.int64)     # perm[c*GPC+k] = graph id
    gslot = np.zeros(G, dtype=np.int64)    # graph id -> c*GPC+k
    for i, g in enumerate(order):
        r, pos = divmod(i, NC)
        c = pos if (r % 2 == 0) else NC - 1 - pos
        perm[c * GPC + r] = g
        gslot[g] = c * GPC + r

    rank = np.arange(N, dtype=np.int64) - start[bat]
    slot_row = gslot[bat] * L + rank       # node -> global slot row
    core_of = gslot[bat] // GPC
    ext_row = slot_row + 16 * core_of      # node -> row in hx/h2x (SLP stride)

    e_core = gslot[bat[dst]] // GPC
    e_slot = slot_row[dst] - e_core * SL   # local dst slot on owning core
    e_w = e_slot // 128

    order = np.lexsort((e_w, e_core))
    src_s, dst_s = src[order], dst[order]
    core_s, w_s, eslot_s = e_core[order], e_w[order], e_slot[order]

    cnt = np.zeros((NC, W), dtype=np.int64)
    np.add.at(cnt, (core_s, w_s), 1)
    B = np.maximum(1, (cnt.max(axis=0) + 127) // 128)
    TB = int(B.sum())
    NEP = TB * 128

    esrc = np.stack([np.full(NEP, c * SLP + SL, dtype=np.int64)
                     for c in range(NC)])             # pad -> own pad row
    dloc = np.zeros((NC, NEP), dtype=np.float32)

    w_off = np.zeros(W + 1, dtype=np.int64)
    np.cumsum(B * 128, out=w_off[1:])

    flat = core_s * W + w_s
    rs = np.searchsorted(flat, np.arange(NC * W))
    re = np.searchsorted(flat, np.arange(NC * W) + 1)
    for c in range(NC):
        for w in range(W):
            a, b = rs[c * W + w], re[c * W + w]
            n = b - a
            o = w_off[w]
            esrc[c, o:o + n] = ext_row[src_s[a:b]]
            dloc[c, o:o + n] = (eslot_s[a:b] % 128).astype(np.float32)

    chunks = []
    b0 = 0
    while b0 < TB:
        nb = min(CH, TB - b0)
        chunks.append((b0, nb))
        b0 += nb

    def build_wrapped(arr):
        parts = []
        for (cb0, nb) in chunks:
            parts.append(_wrap16(arr[cb0 * 128:(cb0 + nb) * 128]))
        return np.concatenate(parts, axis=1)

    ph = np.full((NC, SL), NEG_BIG, dtype=np.float32)
    for c in range(NC):
        for k in range(GPC):
            g = perm[c * GPC + k]
            ph[c, k * L:k * L + counts[g]] = 0.0

    meta = dict(
        N=N, F=F, G=G, GPC=GPC, L=L, SL=SL, SLP=SLP, W=W, TB=TB, perm=perm,
        B=[int(b) for b in B], chunks=chunks, slot_row=slot_row,
        esrc_w=np.stack([build_wrapped(esrc[c]) for c in range(NC)]),
        dloc_t=np.stack([dloc[c].reshape(TB, 128).T.astype(ml_dtypes.bfloat16)
                         for c in range(NC)]),
        ph_t=np.stack([ph[c].reshape(W, 128).T.copy() for c in range(NC)]),
    )
    meta["xmy"] = x_stripes(meta, x)
    return meta


def x_stripes(meta, x):
    """[F, SL] bf16 stripe per core: x rows placed at their slots."""
    import ml_dtypes
    G, L, F, SL = meta["G"], meta["L"], meta["F"], meta["SL"]
    xs = np.zeros((G * L, F), dtype=np.float32)
    xs[meta["slot_row"]] = np.asarray(x, np.float32)
    return np.stack([
        np.ascontiguousarray(xs[c * SL:(c + 1) * SL].T).astype(
            ml_dtypes.bfloat16)
        for c in range(NC)])


# ------------------------------------------------------------- program

def build_program(meta, H, D, D2):
    F, G = meta["F"], meta["G"]
    GPC, L, SL, W, TB = meta["GPC"], meta["L"], meta["SL"], meta["W"], meta["TB"]
    SLP = meta["SLP"]
    NROWX = NC * SLP
    B, chunks = meta["B"], meta["chunks"]
    assert F <= 128 and D == 128

    HD = H * D
    RS1 = ((F + 2 * H + 127) // 128) * 128       # hx row elems: [x | a_src]
    ND1 = H * (F + 1)                            # scatter cols (x*ex | ex) per head
    NB1 = [(k * 512, min((k + 1) * 512, ND1)) for k in range((ND1 + 511) // 512)]
    KD = HD // 128
    assert HD % 128 == 0
    N2 = D2 + 2
    RS2 = ((D2 + 2 + 127) // 128) * 128          # h2x row elems (bf16)
    ND2 = D2 + 1

    nc = bass.Bass(dynamic_dma_scratch_size=DMA_SCRATCH)

    xT_d = nc.declare_dram_parameter("xT", [F, SL], BF16, isOutput=False)
    esrc_d = nc.declare_dram_parameter("esrc", [16, TB * 8], I16, isOutput=False)
    dloc_d = nc.declare_dram_parameter("dloc", [128, TB], BF16, isOutput=False)
    ph_d = nc.declare_dram_parameter("phmask", [128, W], F32, isOutput=False)
    W1_d = nc.declare_dram_parameter("W1", [F, HD], BF16, isOutput=False)
    att1T_d = nc.declare_dram_parameter("att1T", [D, 2 * H], F32, isOutput=False)
    b1_d = nc.declare_dram_parameter("b1", [1, HD], F32, isOutput=False)
    W2_d = nc.declare_dram_parameter("W2", [HD, D2], BF16, isOutput=False)
    att2T_d = nc.declare_dram_parameter("att2T", [D2, 2], F32, isOutput=False)
    b2_d = nc.declare_dram_parameter("b2", [1, D2], F32, isOutput=False)
    fcW_d = nc.declare_dram_parameter("fcW", [D2, D2], F32, isOutput=False)
    fcb_d = nc.declare_dram_parameter("fcb", [1, D2], F32, isOutput=False)
    out_d = nc.declare_dram_parameter("out", [GPC, D2], F32, isOutput=True)

    with tile.TileContext(nc) as tc, ExitStack() as ctx:
        dram = ctx.enter_context(tc.tile_pool(name="dram", bufs=1, space="DRAM"))
        hx = dram.tile([NROWX, RS1], BF16)
        elu1d = dram.tile([SL, HD], BF16)
        xsh = dram.tile([F, SL], BF16)
        xg = dram.tile([NC * F, SL], BF16, addr_space="Shared")
        h2x_shard = dram.tile([SLP, RS2], BF16)
        h2x = dram.tile([NROWX, RS2], BF16, addr_space="Shared")

        const = ctx.enter_context(tc.tile_pool(name="const", bufs=1))
        res = ctx.enter_context(tc.tile_pool(name="res", bufs=1))

        # ---- on-device constants
        iota_f = const.tile([128, 128], BF16)
        nc.gpsimd.iota(iota_f[:], [[1, 128]], channel_multiplier=0,
                       allow_small_or_imprecise_dtypes=True)
        iota_c = const.tile([128, 1], F32)
        nc.gpsimd.iota(iota_c[:], [[0, 1]], channel_multiplier=1,
                       allow_small_or_imprecise_dtypes=True)
        idbf = const.tile([128, 128], BF16)
        nc.vector.tensor_scalar(out=idbf[:], in0=iota_f[:],
                                scalar1=iota_c[:], scalar2=None,
                                op0=OP.is_equal)
        idf32 = const.tile([128, 128], F32)
        nc.vector.tensor_scalar(out=idf32[:], in0=iota_f[:],
                                scalar1=iota_c[:], scalar2=None,
                                op0=OP.is_equal)

        dloc_b = const.tile([128, TB], BF16)
        nc.sync.dma_start(out=dloc_b[:], in_=dloc_d[:])
        dloc_t = const.tile([128, TB], F32)
        nc.vector.tensor_copy(dloc_t[:], dloc_b[:])
        ph_t = const.tile([128, W], F32)
        nc.sync.dma_start(out=ph_t[:], in_=ph_d[:])
        idxt = const.tile([128, TB * 8], I16)
        for g in range(8):
            nc.sync.dma_start(out=idxt[16 * g:16 * (g + 1), :], in_=esrc_d[:])

        b1bc = const.tile([128, HD], BF16)
        b2row = const.tile([1, D2], F32)
        nc.sync.dma_start(out=b2row[:], in_=b2_d[:])
        b2bc = const.tile([128, D2], F32)
        nc.gpsimd.partition_broadcast(b2bc[:], b2row[:])
        fcbrow = const.tile([1, D2], F32)
        nc.sync.dma_start(out=fcbrow[:], in_=fcb_d[:])
        fcbbc = const.tile([128, D2], F32)
        nc.gpsimd.partition_broadcast(fcbbc[:], fcbrow[:])
        fcw_t = const.tile([D2, D2], F32)
        nc.sync.dma_start(out=fcw_t[:], in_=fcW_d[:])

        w1sb = res.tile([F, HD], BF16)
        wsrc = res.tile([F, H], BF16)
        wdst = res.tile([F, H], BF16)
        w2ext = res.tile([128, KD, D2 + 2], BF16)
        adw1 = res.tile([128, W * H], BF16)
        adw2 = res.tile([128, W], BF16)
        out2T = res.tile([128, SL], F32)

        # ---------------- phase A: weight prep (all from shipped raw W)
        pA = ctx.enter_context(tc.tile_pool(name="phA", bufs=2))
        with tc.tile_pool(name="psA", bufs=2, space="PSUM") as psA:
            b1row = pA.tile([1, HD], F32, tag="b1row")
            nc.sync.dma_start(out=b1row[:], in_=b1_d[:])
            b1bcf = pA.tile([128, HD], F32, tag="b1bcf")
            nc.gpsimd.partition_broadcast(b1bcf[:], b1row[:])
            nc.vector.tensor_copy(b1bc[:], b1bcf[:])

            nc.sync.dma_start(out=w1sb[:], in_=W1_d[:])
            att1f = pA.tile([D, 2 * H], F32, tag="att1f")
            nc.sync.dma_start(out=att1f[:], in_=att1T_d[:])
            att1b = pA.tile([D, 2 * H], BF16, tag="att1b")
            nc.vector.tensor_copy(att1b[:], att1f[:])
            # per-head: transpose W1 block, then W1@att columns
            for h in range(H):
                trp = psA.tile([128, F], BF16, tag="trp")
                nc.tensor.transpose(trp[:], w1sb[:, h * 128:(h + 1) * 128],
                                    idbf[0:F, 0:F])
                w1tj = pA.tile([128, F], BF16, tag="w1tj")
                nc.scalar.copy(w1tj[:], trp[:])
                wat = psA.tile([F, 2], F32, tag="wat")
                nc.tensor.matmul(out=wat[:, 0:1], lhsT=w1tj[:],
                                 rhs=att1b[:, h:h + 1], start=True, stop=True)
                nc.tensor.matmul(out=wat[:, 1:2], lhsT=w1tj[:],
                                 rhs=att1b[:, H + h:H + h + 1],
                                 start=True, stop=True)
                nc.vector.tensor_copy(wsrc[:, h:h + 1], wat[:, 0:1])
                nc.vector.tensor_copy(wdst[:, h:h + 1], wat[:, 1:2])

            att2f = pA.tile([D2, 2], F32, tag="att2f")
            nc.sync.dma_start(out=att2f[:], in_=att2T_d[:])
            att2b = pA.tile([D2, 2], BF16, tag="att2b")
            nc.vector.tensor_copy(att2b[:], att2f[:])
            for j in range(KD):
                w2c = pA.tile([128, D2], BF16, tag="w2c")
                nc.sync.dma_start(out=w2c[:],
                                  in_=W2_d[j * 128:(j + 1) * 128, :])
                nc.vector.tensor_copy(w2ext[:, j, 0:D2], w2c[:])
                tr2 = psA.tile([128, 128], BF16, tag="tr2")
                nc.tensor.transpose(tr2[:], w2c[:], idbf[:])
                w2tj = pA.tile([128, 128], BF16, tag="w2tj")
                nc.scalar.copy(w2tj[:], tr2[:])
                w2a = psA.tile([128, 2], F32, tag="w2a")
                nc.tensor.matmul(out=w2a[:], lhsT=w2tj[:], rhs=att2b[:],
                                 start=True, stop=True)
                nc.vector.tensor_copy(w2ext[:, j, D2:D2 + 2], w2a[:])

            padrow = pA.tile([1, RS1], BF16, tag="padrow")
            nc.vector.memset(padrow[:], 0.0)
            nc.vector.memset(padrow[:, F:F + 2 * H].bitcast(F32), NEG_BIG)
            for c in range(NC):
                nc.sync.dma_start(out=hx[c * SLP + SL:c * SLP + SL + 1, :],
                                  in_=padrow[:])
            padrow2 = pA.tile([16, RS2], BF16, tag="padrow2")
            nc.vector.memset(padrow2[:], 0.0)
            nc.vector.memset(padrow2[:, D2:D2 + 2].bitcast(F32), NEG_BIG)
            nc.sync.dma_start(out=h2x_shard[SL:SL + 16, :], in_=padrow2[:])

        # ---------------- phase B0: AllGather x stripes + window a_dst
        with tc.tile_pool(name="phB0", bufs=1) as pB0, \
             tc.tile_pool(name="psB0", bufs=2, space="PSUM") as psB0:
            xmy_t = pB0.tile([F, SL], BF16)
            nc.sync.dma_start(out=xmy_t[:], in_=xT_d[:])
            nc.sync.dma_start(out=xsh[:], in_=xmy_t[:])
            nc.gpsimd.collective_compute(
                "AllGather", OP.bypass,
                replica_groups=[list(range(NC))],
                ins=[xsh[:]],
                outs=[xg[0:NC * F, :]])
            for w in range(W):
                adps = psB0.tile([128, 16], F32, tag="adps")
                nc.tensor.matmul(out=adps[:, 0:H],
                                 lhsT=xmy_t[:, w * 128:(w + 1) * 128],
                                 rhs=wdst[:], start=True, stop=True)
                nc.scalar.copy(adw1[:, w * H:(w + 1) * H], adps[:, 0:H])

        # ---------------- phase B: [x | a_src] rows for all slots (replicated)
        with tc.tile_pool(name="xg", bufs=1) as pxg, \
             tc.tile_pool(name="phB", bufs=6) as pB, \
             tc.tile_pool(name="psB", bufs=2, space="PSUM") as psB:
            xg_t = pxg.tile([F, NC, SL], BF16)
            for c in range(NC):
                nc.sync.dma_start(out=xg_t[:, c, :],
                                  in_=xg[c * F:(c + 1) * F, :])
            for gb in range(NC * W):
                c, lw = divmod(gb, W)
                xt_ps = psB.tile([128, F], BF16, tag="xt")
                nc.tensor.transpose(xt_ps[:],
                                    xg_t[:, c, lw * 128:(lw + 1) * 128],
                                    idbf[0:F, 0:F])
                asps = psB.tile([128, 16], F32, tag="as")
                nc.tensor.matmul(out=asps[:, 0:H],
                                 lhsT=xg_t[:, c, lw * 128:(lw + 1) * 128],
                                 rhs=wsrc[:], start=True, stop=True)
                hrow = pB.tile([128, RS1], BF16, tag="hrow")
                nc.vector.memset(hrow[:, F + 2 * H:RS1], 0.0)
                nc.scalar.copy(hrow[:, 0:F], xt_ps[:])
                nc.vector.tensor_copy(
                    hrow[:, F:F + 2 * H].bitcast(F32), asps[:, 0:H])
                r0 = c * SLP + lw * 128
                nc.sync.dma_start(out=hx[r0:r0 + 128, :], in_=hrow[:])

        # ---------------- edge pass (shared between the two layers)
        _nreg_cache = {}

        def nreg(v):
            if v not in _nreg_cache:
                _nreg_cache[v] = nc.gpsimd.to_reg(v)
            return _nreg_cache[v]

        blk_win = []
        for w in range(W):
            for i in range(B[w]):
                blk_win.append((w, i))

        def edge_pass(layer):
            if layer == 1:
                table, adw = hx, adw1
                ELEM, nd, heads, hd, nbch = RS1, ND1, H, HD, NB1
            else:
                table, adw = h2x, adw2
                ELEM, nd, heads, hd, nbch = RS2, ND2, 1, D2, [(0, ND2)]
            FW = F + 1   # per-head scatter width in layer 1: [x*ex | ex]

            with tc.tile_pool(name=f"gth{layer}", bufs=4) as pG, \
                 tc.tile_pool(name=f"chn{layer}", bufs=2) as pC2, \
                 tc.tile_pool(name=f"spool{layer}", bufs=2 * CH + 2) as pS, \
                 tc.tile_pool(name=f"psw{layer}", bufs=2, space="PSUM") as psW, \
                 tc.tile_pool(name=f"pss{layer}", bufs=1, space="PSUM") as psS, \
                 tc.tile_pool(name=f"psa{layer}", bufs=1, space="PSUM") as psAE, \
                 tc.tile_pool(name=f"pst{layer}", bufs=2 if layer == 2 else 1,
                              space="PSUM") as psT, \
                 tc.tile_pool(name=f"pso{layer}", bufs=1, space="PSUM") as psO, \
                 tc.tile_pool(name=f"nrm{layer}", bufs=2) as pN:

                state = {"w": -1, "ps": None}

                def normalize():
                    w, win_ps = state["w"], state["ps"]
                    rec = pN.tile([128, heads], F32, tag="rec")
                    if layer == 1:
                        den = bass.AP(win_ps.tensor, win_ps[:].offset + F,
                                      [win_ps[:].ap[0], [FW, heads]])
                    else:
                        den = win_ps[:, hd:hd + heads]
                    nc.vector.tensor_scalar_add(rec[:], den, EPS)
                    nc.vector.reciprocal(rec[:], rec[:])
                    odt = BF16 if layer == 1 else F32
                    o1 = pN.tile([128, hd], odt, tag="o1")
                    if layer == 1:
                        # out1 = (sum ex*x) @ W1 per head, then 1/denom
                        wag = pN.tile([128, nd], F32, tag="wag")
                        nc.vector.tensor_copy(wag[:], win_ps[:])
                        for h in range(heads):
                            trp = psT.tile([F, 128], F32, tag="trp")
                            nc.tensor.transpose(
                                trp[:], wag[:, h * FW:h * FW + F], idf32[:])
                            xaT = pN.tile([F, 128], BF16, tag="xaT")
                            nc.scalar.copy(xaT[:], trp[:])
                            o1ps = psO.tile([128, D], F32, tag="o1ps")
                            nc.tensor.matmul(
                                out=o1ps[:], lhsT=xaT[:],
                                rhs=w1sb[:, h * D:(h + 1) * D],
                                start=True, stop=True)
                            nc.scalar.activation(
                                o1[:, h * D:(h + 1) * D], o1ps[:],
                                AF.Copy, scale=rec[:, h:h + 1])
                    else:
                        nc.scalar.activation(o1[:], win_ps[:, 0:hd],
                                             AF.Copy, scale=rec[:, 0:1])
                    bt = b1bc if layer == 1 else b2bc
                    t1 = pN.tile([128, hd], odt, tag="t1")
                    nc.vector.tensor_tensor(t1[:], o1[:], bt[:], OP.add)
                    t2 = pN.tile([128, hd], odt, tag="t2")
                    nc.vector.tensor_scalar_min(t2[:], t1[:], 0.0)
                    e1 = pN.tile([128, hd], odt, tag="e1")
                    nc.scalar.activation(e1[:], t2[:], AF.Exp)
                    r1 = pN.tile([128, hd], odt, tag="r1")
                    nc.scalar.activation(r1[:], t1[:], AF.Relu)
                    el = pN.tile([128, hd], odt, tag="el")
                    nc.vector.scalar_tensor_tensor(
                        out=el[:], in0=e1[:], scalar=-1.0, in1=r1[:],
                        op0=OP.add, op1=OP.add)
                    if layer == 1:
                        nc.sync.dma_start(
                            out=elu1d[w * 128:(w + 1) * 128, :], in_=el[:])
                    else:
                        elm = pN.tile([128, hd], F32, tag="elm")
                        nc.vector.tensor_scalar_add(elm[:], el[:],
                                                    ph_t[:, w:w + 1])
                        tp = psT.tile([128, 128], F32, tag="tp")
                        nc.tensor.transpose(tp[:], elm[:], idf32[:])
                        nc.vector.tensor_copy(out2T[:, w * 128:(w + 1) * 128],
                                              tp[:])

                idx_off = 0
                for (cb0, nbk) in chunks:
                    ne = nbk * 128
                    gt = pG.tile([128, CH, ELEM], BF16, tag="gt")
                    nc.gpsimd.dma_gather(
                        out_ap=gt[:, 0:nbk, :],
                        in_ap=table[:, 0:ELEM],
                        idxs_ap=idxt[:, idx_off:idx_off + nbk * 8],
                        num_idxs=ne, num_idxs_reg=nreg(ne), elem_size=ELEM)
                    idx_off += nbk * 8

                    # per-block a_dst: ae = transpose(S) @ adw[window]
                    ae_all = pC2.tile([128, CH, heads], F32, tag="ae")
                    s_list = []
                    for i in range(nbk):
                        b = cb0 + i
                        w, pos = blk_win[b]
                        s_t = pS.tile([128, 128], BF16, tag="s")
                        nc.vector.tensor_scalar(
                            out=s_t[:], in0=iota_f[:],
                            scalar1=dloc_t[:, b:b + 1], scalar2=None,
                            op0=OP.is_equal)
                        s_list.append(s_t)
                        s2ps = psS.tile([128, 128], BF16, tag="s2ps")
                        nc.tensor.transpose(s2ps[:], s_t[:], idbf[:])
                        s2sb = pS.tile([128, 128], BF16, tag="s2sb")
                        nc.scalar.copy(s2sb[:], s2ps[:])
                        aeps = psAE.tile([128, 16], F32, tag="aeps")
                        nc.tensor.matmul(
                            out=aeps[:, 0:heads], lhsT=s2sb[:],
                            rhs=adw[:, w * heads:(w + 1) * heads],
                            start=True, stop=True)
                        nc.scalar.copy(ae_all[:, i, :], aeps[:, 0:heads])

                    # e = a_src + a_dst ; leakyrelu ; exp  (batched per chunk)
                    if layer == 1:
                        asrc = gt[:, 0:nbk, F:F + 2 * H].bitcast(F32)
                    else:
                        asrc = gt[:, 0:nbk, D2:D2 + 2].bitcast(F32)
                    et = pC2.tile([128, CH * heads], F32, tag="et")
                    nc.vector.tensor_tensor(
                        et[:, 0:nbk * heads], asrc, ae_all[:, 0:nbk, :],
                        OP.add)
                    lk = pC2.tile([128, CH * heads], F32, tag="lk")
                    nc.vector.scalar_tensor_tensor(
                        out=lk[:, 0:nbk * heads], in0=et[:, 0:nbk * heads],
                        scalar=NEG_SLOPE, in1=et[:, 0:nbk * heads],
                        op0=OP.mult, op1=OP.max)
                    exf = pC2.tile([128, CH * heads], F32, tag="exf")
                    nc.scalar.activation(exf[:, 0:nbk * heads],
                                         lk[:, 0:nbk * heads], AF.Exp)
                    exb = pC2.tile([128, CH, heads], BF16, tag="exb")
                    nc.vector.tensor_copy(exb[:, 0:nbk, :],
                                          exf[:, 0:nbk * heads])

                    if layer == 1:
                        # xex[p, i, h, :] = [x_p * ex_p^h | ex_p^h]
                        xex = pC2.tile([128, CH, H, FW], BF16, tag="xex")
                        xsrc = bass.AP(gt.tensor, gt[:].offset,
                                       [gt[:].ap[0], [ELEM, nbk], [0, H],
                                        [1, F]])
                        exb4 = bass.AP(exb.tensor, exb[:].offset,
                                       [exb[:].ap[0], [H, nbk], [1, H],
                                        [0, F]])
                        xexm = bass.AP(xex.tensor, xex[:].offset,
                                       [xex[:].ap[0], [H * FW, nbk], [FW, H],
                                        [1, F]])
                        nc.vector.tensor_tensor(xexm, xsrc, exb4, OP.mult)
                        nc.vector.tensor_copy(
                            bass.AP(xex.tensor, xex[:].offset + F,
                                    [xex[:].ap[0], [H * FW, nbk], [FW, H]]),
                            exb[:, 0:nbk, :])
                        rhs_t = xex
                    else:
                        # scale messages in place, append ex column
                        msg4 = bass.AP(gt.tensor, gt[:].offset,
                                       [gt[:].ap[0], [ELEM, nbk], [D, heads],
                                        [1, D]])
                        exb4 = bass.AP(exb.tensor, exb[:].offset,
                                       [exb[:].ap[0], [heads, nbk], [1, heads],
                                        [0, D]])
                        nc.vector.tensor_tensor(msg4, msg4, exb4, OP.mult)
                        nc.vector.tensor_copy(
                            bass.AP(gt.tensor, gt[:].offset + hd,
                                    [gt[:].ap[0], [ELEM, nbk], [1, heads]]),
                            exb[:, 0:nbk, :])
                        rhs_t = gt

                    # scatter matmuls per block
                    for i in range(nbk):
                        b = cb0 + i
                        w, pos = blk_win[b]
                        if w != state["w"]:
                            if state["w"] >= 0:
                                normalize()
                            state["w"] = w
                            state["ps"] = psW.tile([128, nd], F32, tag="winps",
                                                   name="winps")
                        first, last = pos == 0, pos == B[w] - 1
                        stride = H * FW if layer == 1 else ELEM
                        for (c0, c1) in nbch:
                            rhs = bass.AP(
                                rhs_t.tensor,
                                rhs_t[:].offset + i * stride + c0,
                                [rhs_t[:].ap[0], [1, c1 - c0]])
                            nc.tensor.matmul(
                                out=state["ps"][:, c0:c1],
                                lhsT=s_list[i][:],
                                rhs=rhs,
                                start=first, stop=last)
                normalize()

        edge_pass(1)

        # ---------------- phase D: h2_ext = elu1 @ W2ext on my slots
        with tc.tile_pool(name="phD", bufs=4) as pD, \
             tc.tile_pool(name="psD", bufs=2, space="PSUM") as psD:
            for m in range(W):
                eld = pD.tile([128, HD], BF16, tag="eld")
                nc.sync.dma_start(out=eld[:],
                                  in_=elu1d[m * 128:(m + 1) * 128, :])
                elT = pD.tile([128, KD, 128], BF16, tag="elT")
                for j in range(KD):
                    tpj = psD.tile([128, 128], BF16, tag="tpj")
                    nc.tensor.transpose(tpj[:], eld[:, j * 128:(j + 1) * 128],
                                        idbf[:])
                    nc.vector.tensor_copy(elT[:, j, :], tpj[:])
                h2ps = psD.tile([128, N2], F32, tag="h2ps")
                for j in range(KD):
                    nc.tensor.matmul(
                        out=h2ps[:],
                        lhsT=elT[:, j, :],
                        rhs=w2ext[:, j, :],
                        start=(j == 0), stop=(j == KD - 1))
                row2 = pD.tile([128, RS2], BF16, tag="row2")
                nc.vector.memset(row2[:], 0.0)
                nc.vector.tensor_copy(row2[:, 0:D2], h2ps[:, 0:D2])
                nc.vector.tensor_copy(
                    row2[:, D2:D2 + 2].bitcast(F32), h2ps[:, D2:D2 + 1])
                nc.sync.dma_start(out=h2x_shard[m * 128:(m + 1) * 128, :],
                                  in_=row2[:])
                nc.scalar.copy(adw2[:, m:m + 1], h2ps[:, D2 + 1:D2 + 2])

            nc.gpsimd.collective_compute(
                "AllGather", OP.bypass,
                replica_groups=[list(range(NC))],
                ins=[h2x_shard[:]],
                outs=[h2x[0:NROWX, :]])

        # ---------------- phase E: layer-2 edge pass
        edge_pass(2)

        # ---------------- phase F: pooling + FC
        with tc.tile_pool(name="phF", bufs=1) as pF, \
             tc.tile_pool(name="psF", bufs=1, space="PSUM") as psF:
            pooled = pF.tile([128, GPC], F32)
            o2v = bass.AP(out2T.tensor, out2T[:].offset,
                          [out2T[:].ap[0], [L, GPC], [1, L]])
            nc.vector.tensor_reduce(pooled[:], o2v,
                                    axis=mybir.AxisListType.X, op=OP.max)
            fcps = psF.tile([GPC, D2], F32)
            nc.tensor.matmul(out=fcps[:], lhsT=pooled[:], rhs=fcw_t[:],
                             start=True, stop=True)
            fco = pF.tile([GPC, D2], F32)
            nc.vector.tensor_tensor(fco[:], fcps[:], fcbbc[0:GPC, :], OP.add)
            fcr = pF.tile([GPC, D2], F32)
            nc.scalar.activation(fcr[:], fco[:], AF.Relu)
            nc.sync.dma_start(out=out_d[:], in_=fcr[:])

    return nc


# ------------------------------------------------------------- cached runner

_PROG_CACHE = {}


def get_program(edge_index, batch, H, D, D2, x_shape):
    """Memoize (meta-geometry, program, jitted executable) on the graph
    structure so repeated kernel() calls skip tracing/scheduling/compiling."""
    ei = np.asarray(edge_index)
    bt = np.asarray(batch)
    key = (ei.shape, bt.shape, x_shape, H, D, D2,
           hash(ei.tobytes()), hash(bt.tobytes()))
    ent = _PROG_CACHE.get(key)
    if ent is None:
        x0 = np.zeros(x_shape, np.float32)
        meta = host_prep(x0, ei, bt)
        nc = build_program(meta, H, D, D2)
        finalize_program(nc)
        ent = {"meta_graph": {k: v for k, v in meta.items() if k != "xmy"},
               "nc": nc, "runner": None}
        _PROG_CACHE[key] = ent
    return ent


def _make_runner(nc):
    """A persistent jitted SPMD executor for `nc` (mirrors
    bass2jax.run_bass_via_pjrt's multi-core path, but reusable)."""
    import jax
    from jax.sharding import Mesh, PartitionSpec
    from jax.experimental.shard_map import shard_map
    from concourse import bass2jax
    from concourse.bass2jax import _bass_exec_p, partition_id_tensor
    import concourse.mybir as mb

    bass2jax.install_neuronx_cc_hook()
    assert nc.dbg_addr is None or not nc.dbg_callbacks
    partition_name = (nc.partition_id_tensor.name
                      if nc.partition_id_tensor else None)
    in_names, out_names, out_avals, zero_shapes = [], [], [], []
    for alloc in nc.m.functions[0].allocations:
        if not isinstance(alloc, mb.MemoryLocationSet):
            continue
        name = alloc.memorylocations[0].name
        if alloc.kind == "ExternalInput":
            if name != partition_name and name != (
                    nc.dbg_addr.name if nc.dbg_addr else None):
                in_names.append(name)
        elif alloc.kind == "ExternalOutput":
            out_names.append(name)
            shape = tuple(alloc.tensor_shape)
            dtype = mb.dt.np(alloc.dtype)
            out_avals.append(jax.core.ShapedArray(shape, dtype))
            zero_shapes.append((shape, dtype))
    n_params = len(in_names)
    all_in = list(in_names) + list(out_names)
    if nc.dbg_addr is not None:
        all_in.append(nc.dbg_addr.name)
    if partition_name is not None:
        all_in.append(partition_name)

    def _body(*args):
        operands = list(args)
        if nc.dbg_addr is not None:
            operands.append(np.zeros((1, 2), np.uint32))
        if partition_name is not None:
            operands.append(partition_id_tensor())
        outs = _bass_exec_p.bind(
            *operands,
            out_avals=tuple(out_avals),
            in_names=tuple(all_in),
            out_names=tuple(out_names),
            lowering_input_output_aliases=(),
            sim_require_finite=True,
            sim_require_nnan=True,
            nc=nc,
        )
        return tuple(outs)

    devices = jax.devices()[:NC]
    mesh = Mesh(np.asarray(devices), ("core",))
    n_outs = len(out_names)
    sharded = jax.jit(
        shard_map(_body, mesh=mesh,
                  in_specs=(PartitionSpec("core"),) * (n_params + n_outs),
                  out_specs=(PartitionSpec("core"),) * n_outs,
                  check_rep=False),
        donate_argnums=tuple(range(n_params, n_params + n_outs)),
        keep_unused=True,
    )

    def run(in_maps):
        concat_in = [
            np.concatenate([np.asarray(in_maps[c][name]) for c in range(NC)],
                           axis=0)
            for name in in_names]
        concat_zeros = [np.zeros((NC * s[0], *s[1:]), d)
                        for (s, d) in zero_shapes]
        out_arrs = sharded(*concat_in, *concat_zeros)
        return [
            {name: np.asarray(out_arrs[i]).reshape(NC, *out_avals[i].shape)[c]
             for i, name in enumerate(out_names)}
            for c in range(NC)]

    return run


def run_cached(ent, in_maps):
    if ent["runner"] is None:
        ent["runner"] = _make_runner(ent["nc"])
    return ent["runner"](in_maps)


# ------------------------------------------------------------- entry point

def make_in_maps(meta, x, W1, att_src1, att_dst1, b1, W2, att_src2, att_dst2,
                 b2, fc_W, fc_b):
    import ml_dtypes
    shared = {
        "W1": np.asarray(W1, np.float32).astype(ml_dtypes.bfloat16),
        "att1T": np.ascontiguousarray(np.concatenate(
            [np.asarray(att_src1, np.float32).T,
             np.asarray(att_dst1, np.float32).T], axis=1)),
        "b1": np.asarray(b1, np.float32).reshape(1, -1),
        "W2": np.asarray(W2, np.float32).astype(ml_dtypes.bfloat16),
        "att2T": np.ascontiguousarray(np.concatenate(
            [np.asarray(att_src2, np.float32).T,
             np.asarray(att_dst2, np.float32).T], axis=1)),
        "b2": np.asarray(b2, np.float32).reshape(1, -1),
        "fcW": np.asarray(fc_W, np.float32),
        "fcb": np.asarray(fc_b, np.float32).reshape(1, -1),
    }
    in_maps = []
    for c in range(NC):
        m = dict(shared)
        m["xT"] = meta["xmy"][c]
        m["esrc"] = meta["esrc_w"][c]
        m["dloc"] = meta["dloc_t"][c]
        m["phmask"] = meta["ph_t"][c]
        in_maps.append(m)
    return in_maps


def kernel(**inputs):
    apply_patches()

    x = np.asarray(inputs["x"], np.float32)
    att_src1 = np.asarray(inputs["att_src1"], np.float32)
    H, D = att_src1.shape
    D2 = np.asarray(inputs["W2"]).shape[1]

    ent = get_program(inputs["edge_index"], inputs["batch"], H, D, D2,
                      x.shape)
    meta = dict(ent["meta_graph"])
    meta["xmy"] = x_stripes(meta, x)
    in_maps = make_in_maps(
        meta, x, inputs["W1"], att_src1, inputs["att_dst1"], inputs["b1"],
        inputs["W2"], inputs["att_src2"], inputs["att_dst2"], inputs["b2"],
        inputs["fc_W"], inputs["fc_b"])
    results = run_cached(ent, in_maps)
    G = meta["G"]
    out = np.zeros((G, D2), np.float32)
    for c in range(NC):
        rows = np.asarray(results[c]["out"])
        for k in range(meta["GPC"]):
            out[meta["perm"][c * meta["GPC"] + k]] = rows[k]
    return out
